# revision 2
# baseline (speedup 1.0000x reference)
"""Trainium2 Bass kernel for nn_ColorTransform: per-pixel degree-3 polynomial
color transform  y[b,c,h,w] = bias[c] + sum_f weight[f,c] * mono_f(x[b,:,h,w]).

Strategy (pure data parallel over batch across 8 cores; identical SPMD program):

The 3->19->3 per-pixel map is represented EXACTLY with R=8 runtime-fitted
affine forms L_i = a_i.x + b_i:

    y_c = sum_i cq[i,c] * L_i^3 + cs[i,c] * L_i^2

(8 forms x 10 params = 80 dof >= 60 target coefficients; LM fit on the host
finds an exact representation; forms are rounded to f16 and coefficients
re-solved so weight quantization does not amplify.)

Per-pixel device cost with G=16 pixel-groups per matmul column (R*G = 128
partitions exactly):
  PE  M1: P1 = wm1^T @ X   (block-diag forms)    [128, 1024] PSUM, f16 in
  ACT   : S  = Square(P1)  (f16)                 SBUF
  DVE   : Q  = S * P1      (cube, f16)           SBUF
  PE  M2: P2 = wq^T @ Q + ws^T @ S (accum)       col-tiled 128x64: the two
          units of a pair land in PSUM halves (48 rows at 0 / 64)
  ACT/DVE: ONE [112,1024] copy-out per pair      SBUF f16
Output y is stored f16 (host upcasts) halving write traffic.
HW=512*512 = 32 groups of ND=8192 per plane -> 4 uniform chunks of gpb=8,
no tail. 8 compute units (NCMP=1024 cols) per chunk = 4 pairs.
"""
import os

import numpy as np

import concourse.bass as bass
import concourse.tile as tile
from concourse import bacc, mybir
from concourse.bass_utils import run_bass_kernel_spmd

# ---------------------------------------------------------------- constants
B, C, H, W = 16, 3, 512, 512
HW = H * W
NCORES = 8
BPC = B // NCORES          # batches per core = 2
R = 8                      # affine forms per pixel-group
GPB = 8                    # groups per batch per chunk
NG = BPC * GPB             # pixel groups per matmul column = 16
ND = 8192                  # columns per chunk (per group)
NCMP = int(os.environ.get("KV2_NCMP", "512"))    # compute columns per unit
S2LAG = int(os.environ.get("KV2_S2LAG", "1"))    # stage1->stage2 separation
PSBUFS = int(os.environ.get("KV2_PSBUFS", "4"))  # P1 PSUM pool depth
P2BUFS = int(os.environ.get("KV2_P2BUFS", "4"))  # P2 PSUM pool depth
WIDE_P2 = int(os.environ.get("KV2_WIDE", "1"))   # pairs per out-copy
SQBUFS = int(os.environ.get("KV2_SQBUFS", "6"))  # S/Q SBUF pool depth
SPLIT = ND // NCMP         # 8 units per chunk
CHUNKS = HW // (GPB * ND)  # 4, exact (no tail)
KX = 3 * NG + 1            # X tile rows = 49
RW = R * NG                # P1 rows = 128
OW = 3 * NG                # M2 weight columns = 48
OWB = 3 * GPB              # output rows per (unit,batch) quadrant = 24
NHALF = SPLIT // 2 * NCMP  # 4096 columns per half-chunk

MONOMIALS = [
    (1,0,0),(0,1,0),(0,0,1),
    (2,0,0),(1,1,0),(1,0,1),(0,2,0),(0,1,1),(0,0,2),
    (3,0,0),(2,1,0),(2,0,1),(1,2,0),(1,1,1),(1,0,2),(0,3,0),(0,2,1),(0,1,2),(0,0,3),
]


# ---------------------------------------------------------------- host math
def _target_vals(weight, bias, X):
    mono = np.stack([X[:, 0]**p * X[:, 1]**q * X[:, 2]**r
                     for (p, q, r) in MONOMIALS], 1)
    return mono @ np.asarray(weight, np.float64) + np.asarray(bias, np.float64)


def _grid(n):
    g = (np.arange(n) + 0.5) / n
    return np.stack(np.meshgrid(g, g, g, indexing="ij"), -1).reshape(-1, 3)


def _lm(resid, jac, p0, max_nfev=300):
    """Minimal Levenberg-Marquardt (numpy-only scipy fallback)."""
    class _R:
        pass
    p = np.asarray(p0, np.float64)
    r = resid(p)
    cost = 0.5 * r @ r
    lam = 1e-3
    for _ in range(max_nfev):
        J = jac(p)
        g = J.T @ r
        H = J.T @ J
        step_ok = False
        for _try in range(8):
            try:
                dp = np.linalg.solve(H + lam * np.diag(np.diag(H) + 1e-12),
                                     -g)
            except np.linalg.LinAlgError:
                lam *= 10; continue
            p2 = p + dp
            r2 = resid(p2)
            c2 = 0.5 * r2 @ r2
            if np.isfinite(c2) and c2 < cost:
                p, r, cost = p2, r2, c2
                lam = max(lam * 0.3, 1e-10)
                step_ok = True
                break
            lam = min(lam * 4.0, 1e8)
        if not step_ok or cost < 1e-28:
            break
    out = _R(); out.x = p; out.cost = cost
    return out


def _fit_forms(weight, bias):
    """LM fit of R affine forms (cubes+squares basis) to the target poly.
    Returns a16 [R,3], b16 [R] (f16-rounded), cq, cs [R,3] f32 re-solved."""
    try:
        from scipy.optimize import least_squares
    except ImportError:
        least_squares = None

    Xf = _grid(9)
    Tf = _target_vals(weight, bias, Xf)
    scale = max(np.abs(Tf).max(), 1e-9)

    def unpack(p):
        a = p[:3*R].reshape(R, 3); b = p[3*R:4*R]
        cq = p[4*R:7*R].reshape(R, 3); cs = p[7*R:10*R].reshape(R, 3)
        return a, b, cq, cs

    def model(p, X):
        a, b, cq, cs = unpack(p)
        L = X @ a.T + b
        return (L**3) @ cq + (L**2) @ cs

    def resid(p):
        return (model(p, Xf) - Tf).ravel()

    def jac(p):
        a, b, cq, cs = unpack(p)
        N = Xf.shape[0]
        L = Xf @ a.T + b
        J = np.zeros((N*3, 10*R))
        L2, L3 = L**2, L**3
        g = 3*L2[:, :, None]*cq[None] + 2*L[:, :, None]*cs[None]  # [N,R,3]
        for i in range(R):
            for v in range(3):
                J[:, 3*i+v] = (g[:, i, :] * Xf[:, v, None]).reshape(N*3)
            J[:, 3*R+i] = g[:, i, :].reshape(N*3)
            for c in range(3):
                col = np.zeros((N, 3)); col[:, c] = L3[:, i]
                J[:, 4*R+3*i+c] = col.reshape(N*3)
                col = np.zeros((N, 3)); col[:, c] = L2[:, i]
                J[:, 7*R+3*i+c] = col.reshape(N*3)
        return J

    best = (np.inf, None)
    for seed in range(40):
        rng = np.random.default_rng(seed)
        a0 = rng.normal(size=(R, 3))
        a0 /= np.linalg.norm(a0, axis=1, keepdims=True)
        b0 = rng.uniform(-0.5, 1.0, R)
        L = Xf @ a0.T + b0
        feats = np.concatenate([L**3, L**2], 1)
        C0, *_ = np.linalg.lstsq(feats, Tf, rcond=None)
        p0 = np.concatenate([a0.ravel(), b0, C0[:R].ravel(), C0[R:].ravel()])
        if least_squares is not None:
            r = least_squares(resid, p0, jac=jac, method="trf",
                              max_nfev=300, xtol=1e-15, ftol=1e-15, gtol=1e-15)
        else:
            r = _lm(resid, jac, p0)
        if r.cost < best[0]:
            best = (r.cost, r.x)
        if best[0] < (1e-7 * scale) ** 2:
            break
    assert best[1] is not None and best[0] < (2e-3 * scale) ** 2, \
        f"form fit failed: cost={best[0]:.3e}"

    # refine with a small penalty on coefficient*feature magnitude: shrinks
    # the cancellation amplification so f16 features stay accurate
    lam = 1e-4
    def resid_reg(p):
        a, b, cq, cs = unpack(p)
        L = Xf @ a.T + b
        f3 = np.abs(L**3).max(0); f2 = np.abs(L**2).max(0)
        pen = np.concatenate([(cq * f3[:, None]).ravel(),
                              (cs * f2[:, None]).ravel()])
        return np.concatenate([resid(p), np.sqrt(lam) * pen])
    if least_squares is not None:
        r = least_squares(resid_reg, best[1], method="trf",
                          max_nfev=400, xtol=1e-15, ftol=1e-15, gtol=1e-15)
    else:
        def jac_reg(p):
            eps = 1e-6
            r0 = resid_reg(p)
            J = np.zeros((r0.size, p.size))
            for i in range(p.size):
                pp = p.copy(); pp[i] += eps
                J[:, i] = (resid_reg(pp) - r0) / eps
            return J
        r = _lm(resid_reg, jac_reg, best[1], max_nfev=60)
    fit_err = np.abs(model(r.x, Xf) - Tf).max()
    if fit_err < 1.5e-3 * scale:
        best = (r.cost, r.x)

    a, b, _, _ = unpack(best[1])
    a16 = a.astype(np.float16).astype(np.float64)
    b16 = b.astype(np.float16).astype(np.float64)
    # re-solve coefficients for the f16-rounded forms (finer grid)
    Xv = _grid(17)
    Tv = _target_vals(weight, bias, Xv)
    L = Xv @ a16.T + b16
    feats = np.concatenate([L**3, L**2], 1)
    Cf, *_ = np.linalg.lstsq(feats, Tv, rcond=None)
    err = np.abs(feats @ Cf - Tv).max()
    assert err < 5e-3 * scale, f"rounded-form residual too large: {err:.3e}"
    return (a16.astype(np.float16), b16.astype(np.float16),
            Cf[:R].astype(np.float32), Cf[R:].astype(np.float32))


# v-major row maps -----------------------------------------------------------
# X rows:  0 = ones; 1 + b*(3*GPB) + v*GPB + g
# P1 rows: b*(R*GPB) + i*GPB + g
# P2/O rows (per half): b*(3*GPB) + c*GPB + g

def _lhs1(a16, b16):
    m = np.zeros((KX, RW), np.float32)
    for b in range(BPC):
        for g in range(GPB):
            for i in range(R):
                col = b*R*GPB + i*GPB + g
                m[0, col] = b16[i]
                for v in range(C):
                    m[1 + b*3*GPB + v*GPB + g, col] = a16[i, v]
    return m.astype(np.float16)


def _lhs2(coeff):
    m = np.zeros((RW, OW), np.float32)
    for b in range(BPC):
        for g in range(GPB):
            for i in range(R):
                for c in range(C):
                    m[b*R*GPB + i*GPB + g, b*3*GPB + c*GPB + g] = coeff[i, c]
    return m


# ---------------------------------------------------------------- bass build
_NC_CACHE = {}


def build_nc(reps=1, chunks=None):
    key = (reps, chunks)
    if key in _NC_CACHE:
        return _NC_CACHE[key]
    f32, f16, f32r = mybir.dt.float32, mybir.dt.float16, mybir.dt.float32r
    nc = bacc.Bacc("TRN2", target_bir_lowering=False, debug=False,
                   num_devices=NCORES)

    xs = nc.dram_tensor("xs", [BPC, C, HW], f16, kind="ExternalInput")
    ones = nc.dram_tensor("ones", [1, ND], f16, kind="ExternalInput")
    wm1 = nc.dram_tensor("wm1", [KX, RW], f16, kind="ExternalInput")
    w2qs = nc.dram_tensor("w2qs", [RW, 2*OW], f16, kind="ExternalInput")
    y = nc.dram_tensor("y", [BPC, C, HW], f16, kind="ExternalOutput")

    nchunks = CHUNKS if chunks is None else chunks

    with tile.TileContext(nc) as tc:
        with (
            tc.tile_pool(name="wpool", bufs=1) as wpool,
            tc.tile_pool(name="xpool", bufs=2) as xpool,
            tc.tile_pool(name="spool", bufs=SQBUFS) as spool,
            tc.tile_pool(name="qpool", bufs=SQBUFS) as qpool,
            tc.tile_pool(name="opool", bufs=2) as opool,
            tc.tile_pool(name="p1pool", bufs=PSBUFS, space="PSUM") as p1pool,
            tc.tile_pool(name="p2pool", bufs=P2BUFS, space="PSUM") as p2pool,
        ):
            def load_w(name, dram, shape, dt_, round_to=None):
                t = wpool.tile(shape, dt_, tag=name)
                nc.sync.dma_start(t[:], dram[:])
                if round_to is None:
                    return t
                tr = wpool.tile(shape, round_to, tag=name + "r")
                nc.vector.tensor_copy(tr[:], t[:])
                return tr

            wm1_sb = load_w("wm1", wm1, [KX, RW], f16)
            w2qs_sb = load_w("w2qs", w2qs, [RW, 2*OW], f16)
            w2 = {"q": w2qs_sb[:, 0:OW], "s": w2qs_sb[:, OW:2*OW]}
            for _ in range(2):
                xt0 = xpool.tile([KX, ND], f16, tag="X")
                nc.sync.dma_start(xt0[0:1, :], ones[:])

            # Software pipeline over units u = (chunk, pair, half): stage1
            # (DMA-in + M1) of unit i is emitted before stage2 (Square/cube)
            # of unit i-1; M2 + paired copy-out fire once a pair's features
            # are emitted, out-DMA at chunk end.
            def stage1(u, st):
                k, p, hf = u
                if p == 0 and hf == 0:
                    xt = xpool.tile([KX, ND], f16, tag="X", name="xt")
                    lo = k * GPB * ND
                    nc.sync.dma_start(
                        xt[1:1+3*GPB*BPC],
                        xs[:, :, lo:lo+GPB*ND].rearrange(
                            "b v (g n) -> b v g n", n=ND))
                    st["xt"] = xt
                cl = (p + (SPLIT // 2)*hf) * NCMP
                p1 = p1pool.tile([RW, NCMP], f32, tag="P1", name="p1")
                for h in range(NCMP // 512):
                    nc.tensor.matmul(p1[:, h*512:(h+1)*512], wm1_sb[:],
                                     st["xt"][:, cl+h*512:cl+(h+1)*512],
                                     start=True, stop=True)
                st[("p1", hf)] = p1

            def stage2(u, st):
                k, p, hf = u
                p1 = st[("p1", hf)]
                s = spool.tile([RW, NCMP], f16, tag="S", name="s")
                nc.scalar.square(s[:], p1[:])
                q = qpool.tile([RW, NCMP], f16, tag="Q", name="q")
                nc.vector.tensor_mul(q[:], s[:], p1[:])
                st[("sq", hf)] = (s, q)

            def stage34(u, st, pair_idx):
                k, p, _ = u
                if p == 0:
                    st["ot"] = opool.tile([64 + OW, NHALF], f16, tag="O",
                                          name="ot")
                ot = st["ot"]
                feats = (st[("sq", 0)], st[("sq", 1)])  # (s, q) per unit
                p2 = p2pool.tile([64 + OW, NCMP], f32, tag="P2", name="p2")
                for h in range(NCMP // 512):
                    hl, hh = h*512, (h+1)*512
                    for wi, start, stop in ((1, True, False), (0, False, True)):
                        wt = w2["q"] if wi == 1 else w2["s"]
                        for uu in range(2):
                            f = feats[uu][wi]
                            qd = 64*uu
                            nc.tensor.matmul(
                                p2[qd:qd+OW, hl:hh], wt,
                                f[:, hl:hh], start=start, stop=stop)
                dst = ot[:, p*NCMP:(p+1)*NCMP]
                if pair_idx % 5 < 3:
                    nc.scalar.copy(dst, p2[:])
                else:
                    nc.vector.tensor_copy(dst, p2[:])
                if p == SPLIT // 2 - 1:
                    lo = k * GPB * ND
                    for hf in range(2):
                        yv = y[:, :, lo:lo+GPB*ND].rearrange(
                            "b c (g t n) -> b c g t n", t=2, n=NHALF)[:, :, :, hf]
                        nc.sync.dma_start(yv, ot[64*hf:64*hf+2*OWB])

            def body():
                units = [(k, p, hf) for k in range(nchunks)
                         for p in range(SPLIT // 2) for hf in range(2)]
                states = {}
                pair_idx = 0

                def st_of(u):
                    return states.setdefault(u[0], {})

                pending = []
                def pump():
                    nonlocal pair_idx
                    uu = pending.pop(0)
                    stage2(uu, st_of(uu))
                    if uu[2] == 1:
                        stage34(uu, st_of(uu), pair_idx)
                        pair_idx += 1
                for u in units:
                    stage1(u, st_of(u))
                    pending.append(u)
                    if len(pending) > S2LAG:
                        pump()
                while pending:
                    pump()

            if reps == 1:
                body()
            else:
                hint = (mybir.EngineType.PE, mybir.EngineType.Activation,
                        mybir.EngineType.DVE, mybir.EngineType.SP)
                with tc.For_i(0, reps, 1, hint_engines=hint):
                    body()

    nc.compile()
    _NC_CACHE[key] = nc
    return nc


_FIT_CACHE = {}


def make_in_maps(x, weight, bias):
    key = (np.asarray(weight).tobytes(), np.asarray(bias).tobytes())
    if key not in _FIT_CACHE:
        _FIT_CACHE[key] = _fit_forms(weight, bias)
    a16, b16, cq, cs = _FIT_CACHE[key]
    shared = {
        "wm1": _lhs1(a16, b16),
        "w2qs": np.concatenate([_lhs2(cq), _lhs2(cs)],
                               axis=1).astype(np.float16),
        "ones": np.ones((1, ND), np.float16),
    }
    x = np.ascontiguousarray(np.asarray(x, np.float16)).reshape(B, C, HW)
    return [dict(shared, xs=x[i*BPC:(i+1)*BPC]) for i in range(NCORES)]


def kernel(x, weight, bias, degree=3, **_unused):
    assert int(degree) == 3, "kernel specialized for degree=3"
    nc = build_nc(reps=1)
    in_maps = make_in_maps(x, weight, bias)
    res = run_bass_kernel_spmd(nc, in_maps, core_ids=list(range(NCORES)))
    out = np.empty((B, C, HW), np.float32)
    for i in range(NCORES):
        out[i*BPC:(i+1)*BPC] = res.results[i]["y"].astype(np.float32)
    return out.reshape(B, C, H, W)


if __name__ == "__main__":
    rng = np.random.default_rng(0)
    x = rng.uniform(0, 1, size=(B, C, H, W)).astype(np.float32)
    weight = rng.normal(size=(19, 3)).astype(np.float32)
    bias = rng.normal(size=(3,)).astype(np.float32)
    got = kernel(x, weight, bias, 3)
    print("ran; out shape", got.shape)
